# revision 15
# baseline (speedup 1.0000x reference)
"""ChannelAttention Trainium2 kernel.

Reference computation (per batch b, group o):
    p_mean[s, c] = mean over (h, w) of x[b, o, s, c, :, :]
    p_max[s, c]  = max  over (h, w) of x[b, o, s, c, :, :]
    out = sigmoid(relu(p_mean @ w1[o].T) @ w2[o].T + relu(p_max @ w1[o].T) @ w2[o].T)
    result[b, o, s, c, 0, 0] = out[s, c]

Strategy: data-parallel over batch B=8 -> one batch per NeuronCore (64 MiB
of x per core; the kernel is HBM-bandwidth bound on streaming x).

Per core, x[b] is viewed as [O*S*C, H*W] = [16384, 1024] and streamed in
2 MiB tiles of [128 partitions, 4*1024] on the sync-engine HWDGE queue
(~409 GB/s sustained, measured). Engine split per 4-block chunk,
balanced against the ~5.1us chunk cadence (measured op costs; the DVE
retires 1 output elem/cycle/lane at 0.96 GHz regardless of dtype, the
activation engine 1 elem/cycle at 1.2 GHz):
  - vector: one 3D max tensor_reduce over all 4 blocks (4.42us) plus,
    on 3 of 4 chunks, the 4th block's sum via scalar_tensor_tensor
    halve+add with accum_out (0.70us; stt reads 2 inputs/cycle and the
    running sum accumulates on the side) -> ~5.0us/chunk,
  - scalar: activation-Copy with accum_out for the other blocks
    (avg 3.25 x 1.43us incl. the accumulator read) plus the MLP
    relu/sigmoid -> ~4.9us/chunk.
The second HWDGE ring (scalar engine) carries only the 3 head-taper x
chunks (so both queues spin up in parallel and first data lands ~2us
earlier), the wdup weight load (emitted after the taper; first needed
~30us in), and the 8 tiny output stores -- none of which head-of-line
block the bulk x stream on sync. GpSimd is entirely unused, which
trims its queue-drain share of the fixed ~9us teardown barrier.
MLP emission is delayed 4 chunks past each group's completion so the
scalar engine never blocks at a relu waiting on vector/tensor.
128 consecutive rows cover 2 s-values x 64 channels, so pooled results
land as [partition = (s%2)*64 + c, column = o*16 + s//2]. The tiny
grouped MLP consumes that layout directly by using block-diagonal
duplicated weights ([[W.T, 0], [0, W.T]], built host-side): one
128x128x16 matmul per (group, pooling path), relu, then two accumulating
16x128x128 matmuls (mean + max paths summed in PSUM), sigmoid, and a
strided store. The 1/1024 mean scale is folded into the mean-path w1
block. Chunk schedule tapers at BOTH ends ([1,1,2] head, [2,1,1] tail)
so reductions start ~10us earlier and the final reduce->MLP->store chain
runs on minimal data.

Measured on the 8-core SPMD setup: ~190.4us (best of 5; host-noise
variance pushes outliers to ~210-228us), vs 228us for the staged
baseline under identical measurement conditions. The stream runs flat
at the single-queue ~409 GB/s cap with both reduce engines ~98% busy;
the residual is the fixed preamble (~7us), DMA ramp (~5us), consumer
drain (~6us), final MLP chain (~3us) and the teardown barrier (~9us).
"""

import numpy as np

import concourse.bacc as bacc
import concourse.bass as bass
import concourse.mybir as mybir
import concourse.tile as tile
from concourse.bass_utils import run_bass_kernel_spmd

B, O, S, C, H, W = 8, 8, 32, 64, 32, 32
HID = C
HWSZ = H * W            # 1024 elements pooled per (b, o, s, c)
ROWS = O * S * C        # 16384 rows per core
RB = 128                # rows per partition block
T = ROWS // RB          # 128 row-blocks per core
JB = 4                  # row-blocks per stream tile (2 MiB DMAs)
SP = S // 2             # 16 pooled columns per group
N_CORES = 8

_CACHE = {}


def _build_nc():
    nc = bacc.Bacc(
        "TRN2", target_bir_lowering=False, debug=False, num_devices=N_CORES
    )
    x = nc.dram_tensor("x", [ROWS, HWSZ], mybir.dt.float32, kind="ExternalInput")
    wdup = nc.dram_tensor(
        "wdup", [128, 3 * O * 128], mybir.dt.float32, kind="ExternalInput"
    )
    out = nc.dram_tensor("out", [O * S, C], mybir.dt.float32, kind="ExternalOutput")

    fp32 = mybir.dt.float32
    AF = mybir.ActivationFunctionType
    ALU = mybir.AluOpType

    with tile.TileContext(nc) as tc:
        with (
            tc.tile_pool(name="xp", bufs=11) as xp,
            tc.tile_pool(name="small", bufs=1) as sp,
            tc.tile_pool(name="psum1", bufs=2, space=bass.MemorySpace.PSUM) as pp1,
            tc.tile_pool(name="psum2", bufs=2, space=bass.MemorySpace.PSUM) as pp2,
        ):
            # Weight load and the tiny output stores ride the scalar-engine
            # HWDGE queue (which only carries the 3 head-taper x chunks), so
            # they never head-of-line block the bulk x stream on sync, and
            # gpsimd stays entirely unused (saves its queue-drain teardown).
            # The load is emitted after the taper chunks (weights are first
            # needed by mlp(0) at ~30us).
            wd = sp.tile([128, 3 * O * 128], fp32)

            pooled_sum = sp.tile([128, T], fp32)
            pooled_max = sp.tile([128, T], fp32)
            junk = sp.tile([128, HWSZ], fp32)
            junk_h = sp.tile([128, HWSZ // 2], fp32)
            h_sb = sp.tile([128, O * 2 * SP], fp32)
            att = sp.tile([SP, O * 128], fp32)

            xv = x.ap().rearrange("(t p) f -> t p f", p=RB)
            ov = out.ap().rearrange("(o j r) c -> o j r c", o=O, j=SP, r=2)

            def mlp(o):
                w1s = wd[:, o * 128 : (o + 1) * 128]
                w1m = wd[:, O * 128 + o * 128 : O * 128 + (o + 1) * 128]
                w2b = wd[:, 2 * O * 128 + o * 128 : 2 * O * 128 + (o + 1) * 128]
                ps1m = pp1.tile([128, SP], fp32, tag="ps1m")
                ps1x = pp1.tile([128, SP], fp32, tag="ps1x")
                nc.tensor.matmul(ps1m[:], w1s, pooled_sum[:, o * SP : (o + 1) * SP])
                nc.tensor.matmul(ps1x[:], w1m, pooled_max[:, o * SP : (o + 1) * SP])
                hm = h_sb[:, o * 2 * SP : o * 2 * SP + SP]
                hx = h_sb[:, o * 2 * SP + SP : (o + 1) * 2 * SP]
                nc.scalar.activation(hm, ps1m[:], AF.Relu)
                nc.scalar.activation(hx, ps1x[:], AF.Relu)
                ps2 = pp2.tile([SP, 128], fp32, tag="ps2")
                nc.tensor.matmul(ps2[:], hm, w2b, start=True, stop=False)
                nc.tensor.matmul(ps2[:], hx, w2b, start=False, stop=True)
                ao = att[:, o * 128 : (o + 1) * 128]
                nc.scalar.activation(ao, ps2[:], AF.Sigmoid)
                nc.scalar.dma_start(ov[o], ao.rearrange("p (r c) -> p r c", r=2))

            # Chunk schedule: short head taper so the reduce pipeline starts
            # on the first 0.5 MiB, 2 MiB bulk, short tail taper so the final
            # reduce->MLP->store chain starts on minimal data.
            chunks = [1, 1, 2] + [4] * 30 + [2, 1, 1]
            assert sum(chunks) == T
            # mlp(o) is emitted DELAY chunks after group o's pooled columns
            # complete, so by the time scalar reaches the relu its matmul
            # inputs are long-computed (emitting it immediately stalled
            # scalar's sum stream ~1.8us per chunk waiting on vector+tensor).
            DELAY = 4
            mlp_at = {}
            cum = 0
            for i, jb in enumerate(chunks):
                cum += jb
                for o in range(O):
                    if cum - jb < (o + 1) * SP <= cum:
                        mlp_at.setdefault(min(i + DELAY, len(chunks) - 1), []).append(o)
            t0 = 0
            nfull = 0
            for i, jb in enumerate(chunks):
                xt = xp.tile([RB, JB, HWSZ], fp32, tag="xt")
                # Head-taper chunks go out on the scalar engine's HWDGE queue
                # (scalar is idle until data arrives): both queues spin up in
                # parallel, so the bulk stream on sync starts ~2 chunks
                # earlier and the consumers get their first data sooner.
                dma_eng = nc.scalar if i < 3 else nc.sync
                dma_eng.dma_start(
                    xt[:, :jb, :],
                    xv[t0 : t0 + jb].transpose([1, 0, 2]),
                )
                if i == 3:
                    nc.scalar.dma_start(wd[:], wdup.ap())
                # Max path: one 3D reduce over the whole chunk on vector.
                nc.vector.tensor_reduce(
                    pooled_max[:, t0 : t0 + jb],
                    xt[:, :jb, :],
                    axis=mybir.AxisListType.X,
                    op=ALU.max,
                )
                # Sum path (the 1/HWSZ mean scale is folded into the w1s
                # weight block host-side): mostly scalar (activation Copy +
                # accum_out); the last block of 3 out of 4 full chunks goes
                # to vector via stt halve+add (2 inputs/cycle, sum free) --
                # that split puts vector at ~5.1us/chunk and scalar at
                # ~4.9us/chunk against the ~5.2us DMA cadence.
                if jb == JB:
                    nfull += 1
                for j in range(jb):
                    t = t0 + j
                    if jb == JB and j == jb - 1 and nfull % 4 != 0:
                        nc.vector.scalar_tensor_tensor(
                            junk_h[:],
                            xt[:, j, 0 : HWSZ // 2],
                            1.0,
                            xt[:, j, HWSZ // 2 : HWSZ],
                            ALU.mult,
                            ALU.add,
                            accum_out=pooled_sum[:, t : t + 1],
                        )
                    else:
                        nc.scalar.activation(
                            junk[:],
                            xt[:, j, :],
                            AF.Copy,
                            accum_out=pooled_sum[:, t : t + 1],
                        )
                for o in mlp_at.get(i, ()):
                    mlp(o)
                t0 = t0 + jb

    nc.compile()
    return nc


def _build_wdup(w1, w2):
    # Three sections of 8 block-diagonal duplicated 128x128 matrices:
    # w1.T scaled by 1/HWSZ (consumes raw row sums -> mean path), w1.T
    # (max path), w2.T.
    wdup = np.zeros((128, 3 * O * 128), dtype=np.float32)
    for o in range(O):
        w1t = np.ascontiguousarray(w1[o].T)  # [C, HID]
        w2t = np.ascontiguousarray(w2[o].T)  # [HID, C]
        for sec, blk in ((0, w1t / HWSZ), (1, w1t), (2, w2t)):
            base = sec * O * 128 + o * 128
            wdup[0:64, base : base + 64] = blk
            wdup[64:128, base + 64 : base + 128] = blk
    return wdup


def kernel(x, w1, w2):
    if "nc" not in _CACHE:
        _CACHE["nc"] = _build_nc()
    nc = _CACHE["nc"]

    x = np.ascontiguousarray(x, dtype=np.float32).reshape(B, ROWS, HWSZ)
    wdup = _build_wdup(
        np.asarray(w1, dtype=np.float32), np.asarray(w2, dtype=np.float32)
    )
    in_maps = [{"x": x[b], "wdup": wdup} for b in range(B)]
    res = run_bass_kernel_spmd(nc, in_maps, core_ids=list(range(N_CORES)))
    out = np.stack([res.results[b]["out"] for b in range(B)])
    return out.reshape(B, O, S, C, 1, 1).astype(np.float32)


# revision 16
# speedup vs baseline: 1.0022x; 1.0022x over previous
"""ChannelAttention Trainium2 kernel.

Reference computation (per batch b, group o):
    p_mean[s, c] = mean over (h, w) of x[b, o, s, c, :, :]
    p_max[s, c]  = max  over (h, w) of x[b, o, s, c, :, :]
    out = sigmoid(relu(p_mean @ w1[o].T) @ w2[o].T + relu(p_max @ w1[o].T) @ w2[o].T)
    result[b, o, s, c, 0, 0] = out[s, c]

Strategy: data-parallel over batch B=8 -> one batch per NeuronCore (64 MiB
of x per core; the kernel is HBM-bandwidth bound on streaming x).

Per core, x[b] is viewed as [O*S*C, H*W] = [16384, 1024] and streamed in
2 MiB tiles of [128 partitions, 4*1024] on the sync-engine HWDGE queue
(~409 GB/s sustained, measured). Engine split per 4-block chunk,
balanced against the ~5.1us chunk cadence (measured op costs; the DVE
retires 1 output elem/cycle/lane at 0.96 GHz regardless of dtype, the
activation engine 1 elem/cycle at 1.2 GHz):
  - vector: one 3D max tensor_reduce over all 4 blocks (4.42us) plus,
    on 3 of 4 chunks, the 4th block's sum via scalar_tensor_tensor
    halve+add with accum_out (0.70us; stt reads 2 inputs/cycle and the
    running sum accumulates on the side) -> ~5.0us/chunk,
  - scalar: activation-Copy with accum_out for the other blocks
    (avg 3.25 x 1.43us incl. the accumulator read) plus the MLP
    relu/sigmoid -> ~4.9us/chunk.
The second HWDGE ring (scalar engine) carries only the 3 head-taper x
chunks (so both queues spin up in parallel and first data lands ~2us
earlier), the wdup weight load (emitted after the taper; first needed
~30us in), and the 8 tiny output stores -- none of which head-of-line
block the bulk x stream on sync. GpSimd is entirely unused, which
trims its queue-drain share of the fixed ~9us teardown barrier.
MLP emission is delayed 4 chunks past each group's completion so the
scalar engine never blocks at a relu waiting on vector/tensor.
128 consecutive rows cover 2 s-values x 64 channels, so pooled results
land as [partition = (s%2)*64 + c, column = o*16 + s//2]. The tiny
grouped MLP consumes that layout directly by using block-diagonal
duplicated weights ([[W.T, 0], [0, W.T]], built host-side): one
128x128x16 matmul per (group, pooling path), relu, then two accumulating
16x128x128 matmuls (mean + max paths summed in PSUM), sigmoid, and a
strided store. The 1/1024 mean scale is folded into the mean-path w1
block. Chunk schedule tapers at BOTH ends ([1,1,2] head, [2,1,1] tail)
so reductions start ~10us earlier and the final reduce->MLP->store chain
runs on minimal data.

Measured on the 8-core SPMD setup: ~190.4us (best of 5; host-noise
variance pushes outliers to ~210-228us), vs 228us for the staged
baseline under identical measurement conditions. The stream runs flat
at the single-queue ~409 GB/s cap with both reduce engines ~98% busy;
the residual is the fixed preamble (~7us), DMA ramp (~5us), consumer
drain (~6us), final MLP chain (~3us) and the teardown barrier (~9us).
"""

import numpy as np

import concourse.bacc as bacc
import concourse.bass as bass
import concourse.mybir as mybir
import concourse.tile as tile
from concourse.bass_utils import run_bass_kernel_spmd

B, O, S, C, H, W = 8, 8, 32, 64, 32, 32
HID = C
HWSZ = H * W            # 1024 elements pooled per (b, o, s, c)
ROWS = O * S * C        # 16384 rows per core
RB = 128                # rows per partition block
T = ROWS // RB          # 128 row-blocks per core
JB = 4                  # row-blocks per stream tile (2 MiB DMAs)
SP = S // 2             # 16 pooled columns per group
N_CORES = 8

_CACHE = {}


def _build_nc():
    nc = bacc.Bacc(
        "TRN2", target_bir_lowering=False, debug=False, num_devices=N_CORES
    )
    x = nc.dram_tensor("x", [ROWS, HWSZ], mybir.dt.float32, kind="ExternalInput")
    wdup = nc.dram_tensor(
        "wdup", [128, 3 * O * 128], mybir.dt.float32, kind="ExternalInput"
    )
    out = nc.dram_tensor("out", [O * S, C], mybir.dt.float32, kind="ExternalOutput")

    fp32 = mybir.dt.float32
    AF = mybir.ActivationFunctionType
    ALU = mybir.AluOpType

    with tile.TileContext(nc) as tc:
        with (
            tc.tile_pool(name="xp", bufs=11) as xp,
            tc.tile_pool(name="small", bufs=1) as sp,
            tc.tile_pool(name="psum1", bufs=1, space=bass.MemorySpace.PSUM) as pp1,
            tc.tile_pool(name="psum2", bufs=1, space=bass.MemorySpace.PSUM) as pp2,
        ):
            # Weight load and the tiny output stores ride the scalar-engine
            # HWDGE queue (which only carries the 3 head-taper x chunks), so
            # they never head-of-line block the bulk x stream on sync, and
            # gpsimd stays entirely unused (saves its queue-drain teardown).
            # The load is emitted after the taper chunks (weights are first
            # needed by mlp(0) at ~30us).
            wd = sp.tile([128, 3 * O * 128], fp32)

            pooled_sum = sp.tile([128, T], fp32)
            pooled_max = sp.tile([128, T], fp32)
            junk = sp.tile([128, HWSZ], fp32)
            junk_h = sp.tile([128, HWSZ // 2], fp32)
            h_sb = sp.tile([128, O * 2 * SP], fp32)
            att = sp.tile([SP, O * 128], fp32)

            xv = x.ap().rearrange("(t p) f -> t p f", p=RB)
            ov = out.ap().rearrange("(o j r) c -> o j r c", o=O, j=SP, r=2)

            # PSUM tiles allocated once and reused by all 8 mlp() calls --
            # consecutive groups are ~20us apart so the WAR deps (next
            # matmul waits for the previous relu/sigmoid read) never stall;
            # fewer logical tiles also shortens the teardown sem chain.
            ps1m = pp1.tile([128, SP], fp32, tag="ps1m")
            ps1x = pp1.tile([128, SP], fp32, tag="ps1x")
            ps2 = pp2.tile([SP, 128], fp32, tag="ps2")

            def mlp(o):
                w1s = wd[:, o * 128 : (o + 1) * 128]
                w1m = wd[:, O * 128 + o * 128 : O * 128 + (o + 1) * 128]
                w2b = wd[:, 2 * O * 128 + o * 128 : 2 * O * 128 + (o + 1) * 128]
                nc.tensor.matmul(ps1m[:], w1s, pooled_sum[:, o * SP : (o + 1) * SP])
                nc.tensor.matmul(ps1x[:], w1m, pooled_max[:, o * SP : (o + 1) * SP])
                hm = h_sb[:, o * 2 * SP : o * 2 * SP + SP]
                hx = h_sb[:, o * 2 * SP + SP : (o + 1) * 2 * SP]
                nc.scalar.activation(hm, ps1m[:], AF.Relu)
                nc.scalar.activation(hx, ps1x[:], AF.Relu)
                nc.tensor.matmul(ps2[:], hm, w2b, start=True, stop=False)
                nc.tensor.matmul(ps2[:], hx, w2b, start=False, stop=True)
                ao = att[:, o * 128 : (o + 1) * 128]
                nc.scalar.activation(ao, ps2[:], AF.Sigmoid)
                nc.scalar.dma_start(ov[o], ao.rearrange("p (r c) -> p r c", r=2))

            # Chunk schedule: short head taper so the reduce pipeline starts
            # on the first 0.5 MiB, 2 MiB bulk, short tail taper so the final
            # reduce->MLP->store chain starts on minimal data.
            chunks = [1, 1, 2] + [4] * 30 + [2, 1, 1]
            assert sum(chunks) == T
            # mlp(o) is emitted DELAY chunks after group o's pooled columns
            # complete, so by the time scalar reaches the relu its matmul
            # inputs are long-computed (emitting it immediately stalled
            # scalar's sum stream ~1.8us per chunk waiting on vector+tensor).
            DELAY = 4
            mlp_at = {}
            cum = 0
            for i, jb in enumerate(chunks):
                cum += jb
                for o in range(O):
                    if cum - jb < (o + 1) * SP <= cum:
                        mlp_at.setdefault(min(i + DELAY, len(chunks) - 1), []).append(o)
            t0 = 0
            nfull = 0
            for i, jb in enumerate(chunks):
                xt = xp.tile([RB, JB, HWSZ], fp32, tag="xt")
                # Head-taper chunks go out on the scalar engine's HWDGE queue
                # (scalar is idle until data arrives): both queues spin up in
                # parallel, so the bulk stream on sync starts ~2 chunks
                # earlier and the consumers get their first data sooner.
                dma_eng = nc.scalar if i < 3 else nc.sync
                dma_eng.dma_start(
                    xt[:, :jb, :],
                    xv[t0 : t0 + jb].transpose([1, 0, 2]),
                )
                if i == 3:
                    nc.scalar.dma_start(wd[:], wdup.ap())
                # Max path: one 3D reduce over the whole chunk on vector.
                nc.vector.tensor_reduce(
                    pooled_max[:, t0 : t0 + jb],
                    xt[:, :jb, :],
                    axis=mybir.AxisListType.X,
                    op=ALU.max,
                )
                # Sum path (the 1/HWSZ mean scale is folded into the w1s
                # weight block host-side): mostly scalar (activation Copy +
                # accum_out); the last block of 3 out of 4 full chunks goes
                # to vector via stt halve+add (2 inputs/cycle, sum free) --
                # that split puts vector at ~5.1us/chunk and scalar at
                # ~4.9us/chunk against the ~5.2us DMA cadence.
                if jb == JB:
                    nfull += 1
                for j in range(jb):
                    t = t0 + j
                    if jb == JB and j == jb - 1 and nfull % 4 != 0:
                        nc.vector.scalar_tensor_tensor(
                            junk_h[:],
                            xt[:, j, 0 : HWSZ // 2],
                            1.0,
                            xt[:, j, HWSZ // 2 : HWSZ],
                            ALU.mult,
                            ALU.add,
                            accum_out=pooled_sum[:, t : t + 1],
                        )
                    else:
                        nc.scalar.activation(
                            junk[:],
                            xt[:, j, :],
                            AF.Copy,
                            accum_out=pooled_sum[:, t : t + 1],
                        )
                for o in mlp_at.get(i, ()):
                    mlp(o)
                t0 = t0 + jb

    nc.compile()
    return nc


def _build_wdup(w1, w2):
    # Three sections of 8 block-diagonal duplicated 128x128 matrices:
    # w1.T scaled by 1/HWSZ (consumes raw row sums -> mean path), w1.T
    # (max path), w2.T.
    wdup = np.zeros((128, 3 * O * 128), dtype=np.float32)
    for o in range(O):
        w1t = np.ascontiguousarray(w1[o].T)  # [C, HID]
        w2t = np.ascontiguousarray(w2[o].T)  # [HID, C]
        for sec, blk in ((0, w1t / HWSZ), (1, w1t), (2, w2t)):
            base = sec * O * 128 + o * 128
            wdup[0:64, base : base + 64] = blk
            wdup[64:128, base + 64 : base + 128] = blk
    return wdup


def kernel(x, w1, w2):
    if "nc" not in _CACHE:
        _CACHE["nc"] = _build_nc()
    nc = _CACHE["nc"]

    x = np.ascontiguousarray(x, dtype=np.float32).reshape(B, ROWS, HWSZ)
    wdup = _build_wdup(
        np.asarray(w1, dtype=np.float32), np.asarray(w2, dtype=np.float32)
    )
    in_maps = [{"x": x[b], "wdup": wdup} for b in range(B)]
    res = run_bass_kernel_spmd(nc, in_maps, core_ids=list(range(N_CORES)))
    out = np.stack([res.results[b]["out"] for b in range(B)])
    return out.reshape(B, O, S, C, 1, 1).astype(np.float32)
